# revision 30
# baseline (speedup 1.0000x reference)
"""Ragged-sequence multi-head attention (B=16, S=1024, D=512, H=8, DH=64)
for 8 Trainium2 NeuronCores.

Strategy: data-parallel over the batch. The 16 sequences are sorted by
length; the 8 longest go to slot 0 (one per core), the 8 shortest to
slot 1. A single SPMD Bass program processes both slots with per-slot
static loop bounds equal to ceil128(max length in that slot); within a
bound, invalid key positions are masked via a per-partition additive
bias on the exp() activation, and padded query rows are zeroed via a
per-partition multiplicative mask.

Per-core pipeline (per slot, all fp32 / fp32r):
  1. x -> xT (PE transpose via identity)
  2. QT = Wq^T @ x^T, KT likewise (feature-major), V in [s, d] layout
  3. per head-pair, per q-chunk, per k-tile:
       scoresT[k, q] = K^T q   (row-packed head pair on the PE array)
       expT = exp(0.125 * scoresT + key_mask_bias)   (ACT engine)
       outT[d, q]  += V^T expT (col-packed head pair)
       denom[., q] += 1^T expT (col-packed head pair, replicated rows)
  4. outT_norm = outT * reciprocal(denom)   (DVE)
  5. out[s, d] = outT_norm^T @ Wo + bo, masked by query validity
"""

import math
import os

import numpy as np

B, S, D = 16, 1024, 512
H, DH = 8, 64
N_CORES = 8
P = 128  # partitions
KC = D // P  # 4 contraction chunks of 128
NT_MAX = S // P  # 8 key tiles max

_BUILD_CACHE: dict = {}


def _ceil128(n: int) -> int:
    return max(P, (int(n) + P - 1) // P * P)


def _build_bass(bounds: tuple[int, int]):
    """Build the Bass program for per-slot bounds (multiples of 128)."""
    from contextlib import ExitStack

    import concourse.bass as bass
    import concourse.mybir as mybir
    import concourse.tile as tile
    from concourse import bacc

    fp32 = mybir.dt.float32
    fp16 = mybir.dt.float16
    Exp = mybir.ActivationFunctionType.Exp
    mult = mybir.AluOpType.mult
    add = mybir.AluOpType.add

    nc = bacc.Bacc("TRN2", target_bir_lowering=False, debug=False)

    xin = nc.dram_tensor("xin", [2, S, D], fp32, kind="ExternalInput").ap()
    ident_d = nc.dram_tensor("ident", [P, P], fp32, kind="ExternalInput").ap()
    kbias_d = nc.dram_tensor("kbias", [2, P, NT_MAX], fp32, kind="ExternalInput").ap()
    qmask_d = nc.dram_tensor("qmask", [2, P, NT_MAX], fp32, kind="ExternalInput").ap()
    w_d = {
        name: nc.dram_tensor(name, [D, D], fp16, kind="ExternalInput").ap()
        for name in ("wq", "wk", "wv", "wo")
    }
    bo_d = nc.dram_tensor("bo", [D], fp32, kind="ExternalInput").ap()
    out_d = nc.dram_tensor("out", [2, S, D], fp32, kind="ExternalOutput").ap()

    NT = [bounds[0] // P, bounds[1] // P]
    QCH = [
        [(qs, min(512, bounds[b] - qs)) for qs in range(0, bounds[b], 512)]
        for b in (0, 1)
    ]

    with ExitStack() as ctx:
        tc = ctx.enter_context(tile.TileContext(nc))
        singles = ctx.enter_context(tc.tile_pool(name="singles", bufs=1))
        wstage_p = ctx.enter_context(tc.tile_pool(name="wstage_p", bufs=2))
        big = ctx.enter_context(tc.tile_pool(name="big", bufs=1))
        xpool = ctx.enter_context(tc.tile_pool(name="xpool", bufs=3))
        epool = ctx.enter_context(tc.tile_pool(name="epool", bufs=3))
        opool = ctx.enter_context(tc.tile_pool(name="opool", bufs=3))
        mmps = ctx.enter_context(tc.tile_pool(name="mmps", bufs=2, space="PSUM"))
        scps = ctx.enter_context(tc.tile_pool(name="scps", bufs=2, space="PSUM"))
        accps = ctx.enter_context(tc.tile_pool(name="accps", bufs=1, space="PSUM"))

        # ---- weights / constants ----
        ones64 = singles.tile([P, DH], fp16)
        nc.vector.memset(ones64, 1.0)
        w_sb = {}
        for name in ("wv", "wq", "wk", "wo"):
            w_sb[name] = singles.tile(
                [P, KC, D], fp16, name=f"w_{name}", tag=f"w_{name}"
            )
        for name in ("wv", "wq"):
            nc.sync.dma_start(
                out=w_sb[name], in_=w_d[name].rearrange("(kc p) n -> p kc n", p=P)
            )

        # ---- phase A first: x DMAs + transposes (no weights needed) ----
        identity = singles.tile([P, P], fp32)
        nc.sync.dma_start(out=identity, in_=ident_d)
        xT = []
        for b in (0, 1):
            xT.append(big.tile([P, KC, bounds[b]], fp16, name=f"xT{b}", tag=f"xT{b}"))
            for st in range(NT[b]):
                x_tile = xpool.tile([P, D], fp32, tag="x_tile")
                nc.sync.dma_start(out=x_tile, in_=xin[b, st * P : (st + 1) * P, :])
                xt_ps = mmps.tile([P, 512], fp32, name="xt_ps", tag="mm")
                for dc in range(KC):
                    nc.tensor.transpose(
                        xt_ps[:, dc * P : (dc + 1) * P],
                        x_tile[:, dc * P : (dc + 1) * P],
                        identity,
                    )
                nc.vector.tensor_copy(
                    out=xT[b][:, :, st * P : (st + 1) * P],
                    in_=xt_ps.rearrange("p (dc c) -> p dc c", dc=KC),
                )

        for name in ("wk", "wo"):
            nc.sync.dma_start(
                out=w_sb[name], in_=w_d[name].rearrange("(kc p) n -> p kc n", p=P)
            )
        kbias_sb = singles.tile([P, 2, NT_MAX], fp32)
        nc.sync.dma_start(out=kbias_sb, in_=kbias_d.rearrange("b p t -> p b t"))
        qmask_sb = singles.tile([P, 2, NT_MAX], fp32)
        nc.sync.dma_start(out=qmask_sb, in_=qmask_d.rearrange("b p t -> p b t"))
        bo_rep = singles.tile([P, D], fp32)
        bo_bcast = bass.AP(tensor=bo_d.tensor, offset=bo_d.offset, ap=[[0, P], [1, D]])
        nc.gpsimd.dma_start(out=bo_rep, in_=bo_bcast)

        # ---- V: slot 0 emitted now; slot 1 rides the filler ----
        V = [
            big.tile([P, NT[b], D], fp16, name=f"V{b}", tag=f"V{b}")
            for b in (0, 1)
        ]

        def v_units(b, st):
            ps_box = []

            def mk_mm(kc):
                def emit():
                    if not ps_box:
                        ps_box.append(
                            mmps.tile([P, 512], fp32, name="v_ps", tag="mm")
                        )
                    nc.tensor.matmul(
                        ps_box[0],
                        xT[b][:, kc, st * P : (st + 1) * P],
                        w_sb["wv"][:, kc, :],
                        start=(kc == 0),
                        stop=(kc == KC - 1),
                    )
                return emit

            def fin():
                nc.vector.tensor_copy(out=V[b][:, st, :], in_=ps_box[0])

            return [mk_mm(kc) for kc in range(KC)] + [fin]

        for st in range(NT[0]):
            for u in v_units(0, st):
                u()

        QT = [
            big.tile([P, KC, bounds[b]], fp16, name=f"QT{b}", tag=f"QT{b}")
            for b in (0, 1)
        ]
        KT = [
            big.tile([P, KC, bounds[b]], fp16, name=f"KT{b}", tag=f"KT{b}")
            for b in (0, 1)
        ]
        outT = [
            big.tile([P, KC, bounds[b]], fp16, name=f"oT{b}", tag=f"oT{b}")
            for b in (0, 1)
        ]

        def qtkt_units(b, hp, dst, wname, qs, w):
            ps_box = []

            def mk_mm(kc):
                def emit():
                    if not ps_box:
                        ps_box.append(
                            mmps.tile([P, 512], fp32, name="qk_ps", tag="mm")
                        )
                    nc.tensor.matmul(
                        ps_box[0][:, :w],
                        w_sb[wname][:, kc, hp * P : (hp + 1) * P],
                        xT[b][:, kc, qs : qs + w],
                        start=(kc == 0),
                        stop=(kc == KC - 1),
                    )
                return emit

            def fin():
                nc.vector.tensor_copy(
                    out=dst[:, hp, qs : qs + w], in_=ps_box[0][:, :w]
                )

            return [mk_mm(kc) for kc in range(KC)] + [fin]

        def outproj_units(b, st):
            ps_box = []

            def mk_mm(hc):
                def emit():
                    if not ps_box:
                        ps_box.append(
                            mmps.tile([P, 512], fp32, name="fo_ps", tag="mm")
                        )
                    nc.tensor.matmul(
                        ps_box[0],
                        outT[b][:, hc, st * P : (st + 1) * P],
                        w_sb["wo"][:, hc, :],
                        start=(hc == 0),
                        stop=(hc == KC - 1),
                    )
                return emit

            def fin():
                fout = opool.tile([P, D], fp32, tag="fout")
                nc.vector.tensor_tensor(fout, ps_box[0], bo_rep, add)
                nc.vector.tensor_scalar_mul(
                    fout, fout, qmask_sb[:, b, st : st + 1]
                )
                nc.sync.dma_start(
                    out=out_d[b, st * P : (st + 1) * P, :], in_=fout
                )

            return [mk_mm(hc) for hc in range(KC)] + [fin]

        def make_scores_exp(b, hp, qs, w, kt):
            s_pair = scps.tile([P, 1024], fp32, name="s_pair", tag="s_pair")
            nc.tensor.matmul(
                s_pair[:, 0:w],
                KT[b][0:DH, hp, kt * P : (kt + 1) * P],
                QT[b][0:DH, hp, qs : qs + w],
                start=True,
                stop=True,
                tile_position=(0, 0),
            )
            nc.tensor.matmul(
                s_pair[:, 512 : 512 + w],
                KT[b][DH:P, hp, kt * P : (kt + 1) * P],
                QT[b][DH:P, hp, qs : qs + w],
                start=True,
                stop=True,
                tile_position=(DH, 0),
            )
            e_pair = epool.tile([P, 2, 512], fp16, name="e_pair", tag="e_pair")
            nc.scalar.activation(
                e_pair[:, :, :w],
                s_pair.rearrange("p (h q) -> p h q", h=2)[:, :, :w],
                Exp,
                bias=kbias_sb[:, b, kt : kt + 1],
                scale=DH**-0.5,
            )
            return e_pair

        def make_pv(state, kt, e_pair):
            b, hp, qs, w = state["key"]
            nt = NT[b]
            if state.get("o_ps") is None:
                state["o_ps"] = accps.tile([P, 512], fp32, name="o_ps", tag="o_ps")
                state["d_ps"] = accps.tile([P, 512], fp32, name="d_ps", tag="d_ps")
            o_ps, d_ps = state["o_ps"], state["d_ps"]
            first, last = kt == 0, kt == nt - 1
            nc.tensor.matmul(
                o_ps[0:DH, :w],
                V[b][:, kt, hp * P : hp * P + DH],
                e_pair[:, 0, :w],
                start=first,
                stop=last,
                tile_position=(0, 0),
                skip_group_check=True,
            )
            nc.tensor.matmul(
                o_ps[DH:P, :w],
                V[b][:, kt, hp * P + DH : (hp + 1) * P],
                e_pair[:, 1, :w],
                start=first,
                stop=last,
                tile_position=(0, DH),
                skip_group_check=True,
            )
            nc.tensor.matmul(
                d_ps[0:DH, :w],
                ones64,
                e_pair[:, 0, :w],
                start=first,
                stop=last,
                tile_position=(0, 0),
                skip_group_check=True,
            )
            nc.tensor.matmul(
                d_ps[DH:P, :w],
                ones64,
                e_pair[:, 1, :w],
                start=first,
                stop=last,
                tile_position=(0, DH),
                skip_group_check=True,
            )
            if last:
                rrep = epool.tile([P, 512], fp32, tag="rrep", bufs=2)
                nc.vector.reciprocal_approx_fast(out=rrep[:, :w], in_=d_ps[:, :w])
                nc.vector.tensor_tensor(
                    outT[b][:, hp, qs : qs + w], o_ps[:, :w], rrep[:, :w], mult
                )
                state["o_ps"] = state["d_ps"] = None

        # ---- choreographed emission ----
        for dst, wname in ((QT[0], "wq"), (KT[0], "wk")):
            for qs, w in QCH[0]:
                for u in qtkt_units(0, 0, dst, wname, qs, w):
                    u()

        blocks = [(0, hp) for hp in range(KC)] + [(1, hp) for hp in range(KC)]
        during_block = [[] for _ in blocks]
        # V for slot 1 drains during slot0 hp0/hp1
        for st in range(NT[1]):
            during_block[st % 2].extend(v_units(1, st))
        for j in range(1, len(blocks)):
            b, hp = blocks[j]
            for dst, wname in ((QT[b], "wq"), (KT[b], "wk")):
                for qs, w in QCH[b]:
                    during_block[j - 1].extend(
                        qtkt_units(b, hp, dst, wname, qs, w)
                    )
        # slot-0 output projection rides along slot-1's attention blocks
        s1_blocks = list(range(KC, 2 * KC))
        d0_units = [u for st in range(NT[0]) for u in outproj_units(0, st)]
        per_block = -(-len(d0_units) // len(s1_blocks))
        for i, j in enumerate(s1_blocks):
            during_block[j].extend(d0_units[i * per_block : (i + 1) * per_block])

        # flat pipelined iteration stream; filler paced within its block
        iter_list = []
        for i, (b, hp) in enumerate(blocks):
            n_in_block = len(QCH[b]) * NT[b]
            c = 0
            for qs, w in QCH[b]:
                for kt in range(NT[b]):
                    iter_list.append((i, b, hp, qs, w, kt, n_in_block - c))
                    c += 1
        filler: list = []
        extended = set()
        chunk_states: dict = {}
        pending = None
        for i, b, hp, qs, w, kt, left_in_block in iter_list:
            if i not in extended:
                extended.add(i)
                filler.extend(during_block[i])
            key = (b, hp, qs)
            if key not in chunk_states:
                chunk_states[key] = {"key": (b, hp, qs, w)}
            e_pair = make_scores_exp(b, hp, qs, w, kt)
            if pending is not None:
                make_pv(*pending)
            pending = (chunk_states[key], kt, e_pair)
            if filler:
                k = -(-len(filler) // max(left_in_block, 1))
                for _ in range(min(k, len(filler))):
                    filler.pop(0)()
        make_pv(*pending)
        while filler:
            filler.pop(0)()

        # slot-1 output projection (tail)
        for st in range(NT[1]):
            for u in outproj_units(1, st):
                u()

    nc.compile()
    return nc


def _get_program(bounds: tuple[int, int]):
    key = bounds
    if key not in _BUILD_CACHE:
        _BUILD_CACHE[key] = _build_bass(bounds)
    return _BUILD_CACHE[key]


def kernel(x, seq_lens, Wq, Wk, Wv, Wo, bo) -> np.ndarray:
    from concourse.bass_utils import run_bass_kernel_spmd

    x = np.ascontiguousarray(np.asarray(x, dtype=np.float32))
    seq_lens_np = np.asarray(seq_lens, dtype=np.int32)
    Wq = np.ascontiguousarray(np.asarray(Wq, dtype=np.float32))
    Wk = np.ascontiguousarray(np.asarray(Wk, dtype=np.float32))
    Wv = np.ascontiguousarray(np.asarray(Wv, dtype=np.float32))
    Wo = np.ascontiguousarray(np.asarray(Wo, dtype=np.float32))
    bo = np.ascontiguousarray(np.asarray(bo, dtype=np.float32))

    # Sort sequences by length: longest 8 -> slot 0, rest -> slot 1.
    order = np.argsort(-seq_lens_np, kind="stable")
    slot_seqs = [order[:N_CORES], order[N_CORES:]]
    bounds = tuple(int(_ceil128(seq_lens_np[s].max())) for s in slot_seqs)

    nc = _get_program(bounds)

    # Per-partition masks laid out as [slot, p, tile]: position t*128+p.
    pos = (np.arange(NT_MAX)[None, :] * P + np.arange(P)[:, None]).astype(np.int32)
    in_maps = []
    for c in range(N_CORES):
        seq_pair = [int(slot_seqs[0][c]), int(slot_seqs[1][c])]
        xin = np.stack([x[seq_pair[0]], x[seq_pair[1]]])
        kbias = np.zeros((2, P, NT_MAX), dtype=np.float32)
        qmask = np.zeros((2, P, NT_MAX), dtype=np.float32)
        for slot, seq in enumerate(seq_pair):
            valid = pos < int(seq_lens_np[seq])
            kbias[slot] = np.where(valid, 0.0, -60.0)
            qmask[slot] = valid.astype(np.float32)
        in_maps.append(
            {
                "xin": xin,
                "ident": np.eye(P, dtype=np.float32),
                "kbias": kbias,
                "qmask": qmask,
                "wq": Wq.astype(np.float16),
                "wk": Wk.astype(np.float16),
                "wv": Wv.astype(np.float16),
                "wo": Wo.astype(np.float16),
                "bo": bo,
            }
        )

    trace = bool(int(os.environ.get("KERNEL_TRACE", "0")))
    res = run_bass_kernel_spmd(
        nc, in_maps, core_ids=list(range(N_CORES)), trace=trace
    )
    kernel.last_results = res

    out = np.zeros((B, S, D), dtype=np.float32)
    for c in range(N_CORES):
        out[int(slot_seqs[0][c])] = res.results[c]["out"][0]
        out[int(slot_seqs[1][c])] = res.results[c]["out"][1]
    return out


# revision 31
# speedup vs baseline: 1.0704x; 1.0704x over previous
"""Ragged-sequence multi-head attention (B=16, S=1024, D=512, H=8, DH=64)
for 8 Trainium2 NeuronCores.

Strategy: data-parallel over the batch. The 16 sequences are sorted by
length; the 8 longest go to slot 0 (one per core), the 8 shortest to
slot 1. A single SPMD Bass program processes both slots with per-slot
static loop bounds equal to ceil128(max length in that slot); within a
bound, invalid key positions are masked via a per-partition additive
bias on the exp() activation, and padded query rows are zeroed via a
per-partition multiplicative mask.

Per-core pipeline (per slot, all fp32 / fp32r):
  1. x -> xT (PE transpose via identity)
  2. QT = Wq^T @ x^T, KT likewise (feature-major), V in [s, d] layout
  3. per head-pair, per q-chunk, per k-tile:
       scoresT[k, q] = K^T q   (row-packed head pair on the PE array)
       expT = exp(0.125 * scoresT + key_mask_bias)   (ACT engine)
       outT[d, q]  += V^T expT (col-packed head pair)
       denom[., q] += 1^T expT (col-packed head pair, replicated rows)
  4. outT_norm = outT * reciprocal(denom)   (DVE)
  5. out[s, d] = outT_norm^T @ Wo + bo, masked by query validity
"""

import math
import os

import numpy as np

B, S, D = 16, 1024, 512
H, DH = 8, 64
N_CORES = 8
P = 128  # partitions
KC = D // P  # 4 contraction chunks of 128
NT_MAX = S // P  # 8 key tiles max

_BUILD_CACHE: dict = {}


def _ceil128(n: int) -> int:
    return max(P, (int(n) + P - 1) // P * P)


def _build_bass(bounds: tuple[int, int]):
    """Build the Bass program for per-slot bounds (multiples of 128)."""
    from contextlib import ExitStack

    import concourse.bass as bass
    import concourse.mybir as mybir
    import concourse.tile as tile
    from concourse import bacc

    fp32 = mybir.dt.float32
    fp16 = mybir.dt.float16
    Exp = mybir.ActivationFunctionType.Exp
    mult = mybir.AluOpType.mult
    add = mybir.AluOpType.add

    nc = bacc.Bacc("TRN2", target_bir_lowering=False, debug=False)

    xin = nc.dram_tensor("xin", [2, S, D], fp32, kind="ExternalInput").ap()
    ident_d = nc.dram_tensor("ident", [P, P], fp32, kind="ExternalInput").ap()
    kbias_d = nc.dram_tensor("kbias", [2, P, NT_MAX], fp32, kind="ExternalInput").ap()
    qmask_d = nc.dram_tensor("qmask", [2, P, NT_MAX], fp32, kind="ExternalInput").ap()
    w_d = {
        name: nc.dram_tensor(name, [D, D], fp16, kind="ExternalInput").ap()
        for name in ("wq", "wk", "wv", "wo")
    }
    bo_d = nc.dram_tensor("bo", [D], fp32, kind="ExternalInput").ap()
    out_d = nc.dram_tensor("out", [2, S, D], fp32, kind="ExternalOutput").ap()

    NT = [bounds[0] // P, bounds[1] // P]
    QCH = [
        [(qs, min(512, bounds[b] - qs)) for qs in range(0, bounds[b], 512)]
        for b in (0, 1)
    ]

    with ExitStack() as ctx:
        tc = ctx.enter_context(tile.TileContext(nc))
        singles = ctx.enter_context(tc.tile_pool(name="singles", bufs=1))
        wstage_p = ctx.enter_context(tc.tile_pool(name="wstage_p", bufs=2))
        big = ctx.enter_context(tc.tile_pool(name="big", bufs=1))
        xpool = ctx.enter_context(tc.tile_pool(name="xpool", bufs=4))
        epool = ctx.enter_context(tc.tile_pool(name="epool", bufs=3))
        opool = ctx.enter_context(tc.tile_pool(name="opool", bufs=4))
        mmps = ctx.enter_context(tc.tile_pool(name="mmps", bufs=2, space="PSUM"))
        scps = ctx.enter_context(tc.tile_pool(name="scps", bufs=2, space="PSUM"))
        accps = ctx.enter_context(tc.tile_pool(name="accps", bufs=1, space="PSUM"))

        # ---- weights / constants ----
        ones64 = singles.tile([P, DH], fp16)
        nc.vector.memset(ones64, 1.0)
        w_sb = {}
        for name in ("wv", "wq", "wk", "wo"):
            w_sb[name] = singles.tile(
                [P, KC, D], fp16, name=f"w_{name}", tag=f"w_{name}"
            )
        for name in ("wv", "wq"):
            nc.sync.dma_start(
                out=w_sb[name], in_=w_d[name].rearrange("(kc p) n -> p kc n", p=P)
            )

        # ---- phase A first: x DMAs + transposes (no weights needed) ----
        identity = singles.tile([P, P], fp32)
        nc.sync.dma_start(out=identity, in_=ident_d)
        xT = []
        for b in (0, 1):
            xT.append(big.tile([P, KC, bounds[b]], fp16, name=f"xT{b}", tag=f"xT{b}"))
            for st in range(NT[b]):
                x_tile = xpool.tile([P, D], fp32, tag="x_tile")
                nc.sync.dma_start(out=x_tile, in_=xin[b, st * P : (st + 1) * P, :])
                xt_ps = mmps.tile([P, 512], fp32, name="xt_ps", tag="mm")
                for dc in range(KC):
                    nc.tensor.transpose(
                        xt_ps[:, dc * P : (dc + 1) * P],
                        x_tile[:, dc * P : (dc + 1) * P],
                        identity,
                    )
                nc.vector.tensor_copy(
                    out=xT[b][:, :, st * P : (st + 1) * P],
                    in_=xt_ps.rearrange("p (dc c) -> p dc c", dc=KC),
                )

        for name in ("wk", "wo"):
            nc.sync.dma_start(
                out=w_sb[name], in_=w_d[name].rearrange("(kc p) n -> p kc n", p=P)
            )
        kbias_sb = singles.tile([P, 2, NT_MAX], fp32)
        nc.sync.dma_start(out=kbias_sb, in_=kbias_d.rearrange("b p t -> p b t"))
        qmask_sb = singles.tile([P, 2, NT_MAX], fp32)
        nc.sync.dma_start(out=qmask_sb, in_=qmask_d.rearrange("b p t -> p b t"))
        bo_rep = singles.tile([P, D], fp32)
        bo_bcast = bass.AP(tensor=bo_d.tensor, offset=bo_d.offset, ap=[[0, P], [1, D]])
        nc.gpsimd.dma_start(out=bo_rep, in_=bo_bcast)

        # ---- V: slot 0 emitted now; slot 1 rides the filler ----
        V = [
            big.tile([P, NT[b], D], fp16, name=f"V{b}", tag=f"V{b}")
            for b in (0, 1)
        ]

        def v_units(b, st):
            ps_box = []

            def mk_mm(kc):
                def emit():
                    if not ps_box:
                        ps_box.append(
                            mmps.tile([P, 512], fp32, name="v_ps", tag="mm")
                        )
                    nc.tensor.matmul(
                        ps_box[0],
                        xT[b][:, kc, st * P : (st + 1) * P],
                        w_sb["wv"][:, kc, :],
                        start=(kc == 0),
                        stop=(kc == KC - 1),
                    )
                return emit

            def fin():
                nc.vector.tensor_copy(out=V[b][:, st, :], in_=ps_box[0])

            return [mk_mm(kc) for kc in range(KC)] + [fin]

        for st in range(NT[0]):
            for u in v_units(0, st):
                u()

        QT = [
            big.tile([P, KC, bounds[b]], fp16, name=f"QT{b}", tag=f"QT{b}")
            for b in (0, 1)
        ]
        KT = [
            big.tile([P, KC, bounds[b]], fp16, name=f"KT{b}", tag=f"KT{b}")
            for b in (0, 1)
        ]
        outT = [
            big.tile([P, KC, bounds[b]], fp16, name=f"oT{b}", tag=f"oT{b}")
            for b in (0, 1)
        ]

        def qtkt_units(b, hp, dst, wname, qs, w):
            ps_box = []

            def mk_mm(kc):
                def emit():
                    if not ps_box:
                        ps_box.append(
                            mmps.tile([P, 512], fp32, name="qk_ps", tag="mm")
                        )
                    nc.tensor.matmul(
                        ps_box[0][:, :w],
                        w_sb[wname][:, kc, hp * P : (hp + 1) * P],
                        xT[b][:, kc, qs : qs + w],
                        start=(kc == 0),
                        stop=(kc == KC - 1),
                    )
                return emit

            def fin():
                nc.vector.tensor_copy(
                    out=dst[:, hp, qs : qs + w], in_=ps_box[0][:, :w]
                )

            return [mk_mm(kc) for kc in range(KC)] + [fin]

        def outproj_units(b, st):
            ps_box = []

            def mk_mm(hc):
                def emit():
                    if not ps_box:
                        ps_box.append(
                            mmps.tile([P, 512], fp32, name="fo_ps", tag="mm")
                        )
                    nc.tensor.matmul(
                        ps_box[0],
                        outT[b][:, hc, st * P : (st + 1) * P],
                        w_sb["wo"][:, hc, :],
                        start=(hc == 0),
                        stop=(hc == KC - 1),
                    )
                return emit

            def fin():
                fout = opool.tile([P, D], fp32, tag="fout")
                nc.vector.tensor_tensor(fout, ps_box[0], bo_rep, add)
                nc.vector.tensor_scalar_mul(
                    fout, fout, qmask_sb[:, b, st : st + 1]
                )
                nc.sync.dma_start(
                    out=out_d[b, st * P : (st + 1) * P, :], in_=fout
                )

            return [mk_mm(hc) for hc in range(KC)] + [fin]

        def make_scores_exp(b, hp, qs, w, kt):
            s_pair = scps.tile([P, 1024], fp32, name="s_pair", tag="s_pair")
            nc.tensor.matmul(
                s_pair[:, 0:w],
                KT[b][0:DH, hp, kt * P : (kt + 1) * P],
                QT[b][0:DH, hp, qs : qs + w],
                start=True,
                stop=True,
                tile_position=(0, 0),
            )
            nc.tensor.matmul(
                s_pair[:, 512 : 512 + w],
                KT[b][DH:P, hp, kt * P : (kt + 1) * P],
                QT[b][DH:P, hp, qs : qs + w],
                start=True,
                stop=True,
                tile_position=(DH, 0),
            )
            e_pair = epool.tile([P, 2, 512], fp16, name="e_pair", tag="e_pair")
            nc.scalar.activation(
                e_pair[:, :, :w],
                s_pair.rearrange("p (h q) -> p h q", h=2)[:, :, :w],
                Exp,
                bias=kbias_sb[:, b, kt : kt + 1],
                scale=DH**-0.5,
            )
            return e_pair

        def make_pv(state, kt, e_pair):
            b, hp, qs, w = state["key"]
            nt = NT[b]
            if state.get("o_ps") is None:
                state["o_ps"] = accps.tile([P, 512], fp32, name="o_ps", tag="o_ps")
                state["d_ps"] = accps.tile([P, 512], fp32, name="d_ps", tag="d_ps")
            o_ps, d_ps = state["o_ps"], state["d_ps"]
            first, last = kt == 0, kt == nt - 1
            nc.tensor.matmul(
                o_ps[0:DH, :w],
                V[b][:, kt, hp * P : hp * P + DH],
                e_pair[:, 0, :w],
                start=first,
                stop=last,
                tile_position=(0, 0),
                skip_group_check=True,
            )
            nc.tensor.matmul(
                o_ps[DH:P, :w],
                V[b][:, kt, hp * P + DH : (hp + 1) * P],
                e_pair[:, 1, :w],
                start=first,
                stop=last,
                tile_position=(0, DH),
                skip_group_check=True,
            )
            nc.tensor.matmul(
                d_ps[0:DH, :w],
                ones64,
                e_pair[:, 0, :w],
                start=first,
                stop=last,
                tile_position=(0, 0),
                skip_group_check=True,
            )
            nc.tensor.matmul(
                d_ps[DH:P, :w],
                ones64,
                e_pair[:, 1, :w],
                start=first,
                stop=last,
                tile_position=(0, DH),
                skip_group_check=True,
            )
            if last:
                rrep = epool.tile([P, 512], fp32, tag="rrep", bufs=2)
                nc.vector.reciprocal_approx_fast(out=rrep[:, :w], in_=d_ps[:, :w])
                nc.vector.tensor_tensor(
                    outT[b][:, hp, qs : qs + w], o_ps[:, :w], rrep[:, :w], mult
                )
                state["o_ps"] = state["d_ps"] = None

        # ---- choreographed emission ----
        for dst, wname in ((QT[0], "wq"), (KT[0], "wk")):
            for qs, w in QCH[0]:
                for u in qtkt_units(0, 0, dst, wname, qs, w):
                    u()

        blocks = [(0, hp) for hp in range(KC)] + [(1, hp) for hp in range(KC)]
        during_block = [[] for _ in blocks]
        # V for slot 1 drains during slot0 hp0/hp1
        for st in range(NT[1]):
            during_block[st % 2].extend(v_units(1, st))
        for j in range(1, len(blocks)):
            b, hp = blocks[j]
            for dst, wname in ((QT[b], "wq"), (KT[b], "wk")):
                for qs, w in QCH[b]:
                    during_block[j - 1].extend(
                        qtkt_units(b, hp, dst, wname, qs, w)
                    )
        # slot-0 output projection rides along slot-1's attention blocks
        s1_blocks = list(range(KC, 2 * KC))
        d0_units = [u for st in range(NT[0]) for u in outproj_units(0, st)]
        per_block = -(-len(d0_units) // len(s1_blocks))
        for i, j in enumerate(s1_blocks):
            during_block[j].extend(d0_units[i * per_block : (i + 1) * per_block])

        # flat pipelined iteration stream; filler paced within its block
        iter_list = []
        for i, (b, hp) in enumerate(blocks):
            n_in_block = len(QCH[b]) * NT[b]
            c = 0
            for qs, w in QCH[b]:
                for kt in range(NT[b]):
                    iter_list.append((i, b, hp, qs, w, kt, n_in_block - c))
                    c += 1
        filler: list = []
        extended = set()
        chunk_states: dict = {}
        pending = None
        for i, b, hp, qs, w, kt, left_in_block in iter_list:
            if i not in extended:
                extended.add(i)
                filler.extend(during_block[i])
            key = (b, hp, qs)
            if key not in chunk_states:
                chunk_states[key] = {"key": (b, hp, qs, w)}
            e_pair = make_scores_exp(b, hp, qs, w, kt)
            if pending is not None:
                make_pv(*pending)
            pending = (chunk_states[key], kt, e_pair)
            if filler:
                k = -(-len(filler) // max(left_in_block, 1))
                for _ in range(min(k, len(filler))):
                    filler.pop(0)()
        make_pv(*pending)
        while filler:
            filler.pop(0)()

        # slot-1 output projection (tail)
        for st in range(NT[1]):
            for u in outproj_units(1, st):
                u()

    nc.compile()
    return nc


def _get_program(bounds: tuple[int, int]):
    key = bounds
    if key not in _BUILD_CACHE:
        _BUILD_CACHE[key] = _build_bass(bounds)
    return _BUILD_CACHE[key]


def kernel(x, seq_lens, Wq, Wk, Wv, Wo, bo) -> np.ndarray:
    from concourse.bass_utils import run_bass_kernel_spmd

    x = np.ascontiguousarray(np.asarray(x, dtype=np.float32))
    seq_lens_np = np.asarray(seq_lens, dtype=np.int32)
    Wq = np.ascontiguousarray(np.asarray(Wq, dtype=np.float32))
    Wk = np.ascontiguousarray(np.asarray(Wk, dtype=np.float32))
    Wv = np.ascontiguousarray(np.asarray(Wv, dtype=np.float32))
    Wo = np.ascontiguousarray(np.asarray(Wo, dtype=np.float32))
    bo = np.ascontiguousarray(np.asarray(bo, dtype=np.float32))

    # Sort sequences by length: longest 8 -> slot 0, rest -> slot 1.
    order = np.argsort(-seq_lens_np, kind="stable")
    slot_seqs = [order[:N_CORES], order[N_CORES:]]
    bounds = tuple(int(_ceil128(seq_lens_np[s].max())) for s in slot_seqs)

    nc = _get_program(bounds)

    # Per-partition masks laid out as [slot, p, tile]: position t*128+p.
    pos = (np.arange(NT_MAX)[None, :] * P + np.arange(P)[:, None]).astype(np.int32)
    in_maps = []
    for c in range(N_CORES):
        seq_pair = [int(slot_seqs[0][c]), int(slot_seqs[1][c])]
        xin = np.stack([x[seq_pair[0]], x[seq_pair[1]]])
        kbias = np.zeros((2, P, NT_MAX), dtype=np.float32)
        qmask = np.zeros((2, P, NT_MAX), dtype=np.float32)
        for slot, seq in enumerate(seq_pair):
            valid = pos < int(seq_lens_np[seq])
            kbias[slot] = np.where(valid, 0.0, -60.0)
            qmask[slot] = valid.astype(np.float32)
        in_maps.append(
            {
                "xin": xin,
                "ident": np.eye(P, dtype=np.float32),
                "kbias": kbias,
                "qmask": qmask,
                "wq": Wq.astype(np.float16),
                "wk": Wk.astype(np.float16),
                "wv": Wv.astype(np.float16),
                "wo": Wo.astype(np.float16),
                "bo": bo,
            }
        )

    trace = bool(int(os.environ.get("KERNEL_TRACE", "0")))
    res = run_bass_kernel_spmd(
        nc, in_maps, core_ids=list(range(N_CORES)), trace=trace
    )
    kernel.last_results = res

    out = np.zeros((B, S, D), dtype=np.float32)
    for c in range(N_CORES):
        out[int(slot_seqs[0][c])] = res.results[c]["out"][0]
        out[int(slot_seqs[1][c])] = res.results[c]["out"][1]
    return out
